# revision 34
# baseline (speedup 1.0000x reference)
"""IntraAttention Trainium2 kernel, 8-core SPMD, mixed fp8/fp16.

Reference computation (N=4096 rows, d=1024):
    Q = X @ Wq.T + bq ; K = X @ Wk.T + bk ; V = X @ Wv.T + bv
    alpha = softmax(Q @ K.T / sqrt(d), axis=1)
    V_ = alpha @ V
    x = concat([V_, Q], axis=1)              # [N, 2d]
    x1 = x @ Wl.T + bl                        # [N, d]
    h = x @ Wa.T + ba                         # [N, 2d]
    out = x1 * (h[:, :d] * sigmoid(h[:, d:]))

Sharding: rows of X are sharded across 8 cores (512 rows each). Q stays
local; K and V shards are all-gathered in fp8 (one gather each --
every collective op carries ~8us fixed cost on the serial CC stream and
the whole stream is gated by a ~40-60us kernel-entry barrier that
absorbs cross-core launch skew). The Q-side fp16 projections (Q, and
the Q-halves of x1/h_a/h_b) run as gap fillers under that barrier+AG
window.

Precision: softmax averaging makes the whole attention path noise
immune, so K/V projections, scores, exp, and alpha@V run as fp8e4
DoubleRow matmuls (2x PE throughput, 256-deep contraction per
instruction). V_ is tiny relative to Q, so the V-halves of the
x1/h projections are fp8 DoubleRow too. Only the Q projection and the
Q-halves of x1/h_a/h_b stay fp16 (they dominate output accuracy).
All matmuls accumulate fp32 in PSUM.

DMA discipline: every weight class gets dedicated SBUF tiles whose
loads are all issued up front (the hardware DMA queues execute
in-order, so a single load waiting on a recycled buffer would block
everything queued behind it), queues are specialized (sync: X + fp16
weights + even K/V attention tiles; scalar: fp8 proj weights, biases,
collective staging, activations; gpsimd: collective triggers + odd
attention tiles), and all transfers move >=1KB contiguous per
partition.
"""

import numpy as np
import ml_dtypes

import concourse.bass as bass
import concourse.bacc as bacc
import concourse.tile as tile
import concourse.bass_utils as bass_utils
from concourse import mybir

P = 128            # partitions
D = 1024           # model dim
N = 4096           # rows
NCORES = 8
R = N // NCORES    # rows per core = 512
HR = R // 2        # half of the local rows = 256
DC = D // P        # d chunks = 8
DP = DC // 2       # d chunk pairs = 4
NK = N // P        # key tiles = 32
NKP = NK // 2      # key tile pairs = 16
TD = 2 * D         # 2048
TDC = TD // P      # 16

F32 = mybir.dt.float32
F16 = mybir.dt.float16
F8 = mybir.dt.float8e4
DRM = mybir.MatmulPerfMode.DoubleRow
F8NP = ml_dtypes.float8_e4m3

RG = [list(range(NCORES))]

# key-tile visit order; kt pairs (i=2t, 2t+1) pair exp slots with V rows
KT_ORDER = [(rr, ss) for ss in range(4) for rr in range(NCORES)]


def build_nc():
    nc = bacc.Bacc(
        "TRN2",
        target_bir_lowering=False,
        debug=False,
        num_devices=NCORES,
    )

    # ---- per-core I/O (host pre-arranged layouts, see make_in_maps) ----
    xt8 = nc.dram_tensor("xt8", [P, DP * 2 * R], F8, kind="ExternalInput")
    xt16 = nc.dram_tensor("xt16", [P, DC * R], F16, kind="ExternalInput")
    wq16 = nc.dram_tensor("wq16", [P, DC * D], F16, kind="ExternalInput")
    wk8 = nc.dram_tensor("wk8", [P, DP * 2 * D], F8, kind="ExternalInput")
    wv8 = nc.dram_tensor("wv8", [P, DP * 2 * D], F8, kind="ExternalInput")
    wlq16 = nc.dram_tensor("wlq16", [P, DC * D], F16, kind="ExternalInput")
    waq16 = nc.dram_tensor("waq16", [P, DC * D], F16, kind="ExternalInput")
    wbq16 = nc.dram_tensor("wbq16", [P, DC * D], F16, kind="ExternalInput")
    wlv8 = nc.dram_tensor("wlv8", [P, DP * 2 * D], F8, kind="ExternalInput")
    wav8 = nc.dram_tensor("wav8", [P, DP * 2 * D], F8, kind="ExternalInput")
    wbv8 = nc.dram_tensor("wbv8", [P, DP * 2 * D], F8, kind="ExternalInput")
    bq = nc.dram_tensor("bq", [P, DC], F32, kind="ExternalInput")
    bk = nc.dram_tensor("bk", [P, DC], F32, kind="ExternalInput")
    bvb = nc.dram_tensor("bvb", [P, D], F32, kind="ExternalInput")    # bv bcast
    bl = nc.dram_tensor("bl", [P, DC], F32, kind="ExternalInput")
    ba = nc.dram_tensor("ba", [P, TDC], F32, kind="ExternalInput")
    out = nc.dram_tensor("out", [D, R], F32, kind="ExternalOutput")   # out_c.T

    # ---- collective buffers (fp8), one gather each for K and V ----
    # K: [p, s, m, n] = K.T[d = m*128+p, key = s*128 + n (local)]
    ktc_d = nc.dram_tensor("ktc_d", [P, 4 * DC * P], F8)
    # V: [p, s, f] = V[row = s*128 + p (local), f]
    vc_d = nc.dram_tensor("vc_d", [P, 4 * D], F8)
    ag_k = nc.dram_tensor("ag_k", [NCORES * P, 4 * DC * P], F8,
                          addr_space="Shared")
    ag_v = nc.dram_tensor("ag_v", [NCORES * P, 4 * D], F8,
                          addr_space="Shared")

    with tile.TileContext(nc) as tc:
        with (
            tc.tile_pool(name="cpool", bufs=1) as cpool,
            tc.tile_pool(name="pspool", bufs=8, space="PSUM") as pspool,
        ):
            bq_t = cpool.tile([P, DC], F32, name="bq_t")
            bk_t = cpool.tile([P, DC], F32, name="bk_t")
            bl_t = cpool.tile([P, DC], F32, name="bl_t")
            ba_t = cpool.tile([P, TDC], F32, name="ba_t")
            bvb_t = cpool.tile([P, D], F32, name="bvb_t")
            # DoubleRow pair-dim stride must be a multiple of 16 elements,
            # so the ones column is padded to 16.
            ones8 = cpool.tile([P, 2, 16], F8, name="ones8")
            nc.vector.memset(ones8, 1.0)
            ones_row = cpool.tile([1, P], F32, name="ones_row")
            nc.vector.memset(ones_row, 1.0)

            with tc.tile_pool(name="qpool", bufs=1) as qpool, \
                 tc.tile_pool(name="qfpool", bufs=1) as qfpool, \
                 tc.tile_pool(name="kvpool", bufs=1) as kvpool:

                # ============ prefetches ============
                # All weight loads go to dedicated tiles and are issued up
                # front so the in-order DMA queues never head-of-line block.
                with tc.tile_pool(name="xwpool", bufs=1) as xwpool, \
                     tc.tile_pool(name="fwpool", bufs=1) as fwpool:
                    # sync queue: X fp8 pairs, X fp16 chunks, Wq, Wl_q, Wb_q
                    x8_t = [xwpool.tile([P, 2, R], F8, name=f"x8_{kp}")
                            for kp in range(DP)]
                    for kp in range(DP):
                        nc.sync.dma_start(
                            x8_t[kp],
                            xt8[:, kp * 2 * R:(kp + 1) * 2 * R]
                            .rearrange("p (j n) -> p j n", j=2))
                    xt_t = [xwpool.tile([P, R], F16, name=f"xt{k}")
                            for k in range(DC)]
                    for k in range(DC):
                        nc.sync.dma_start(xt_t[k], xt16[:, k * R:(k + 1) * R])
                    wq_t = [xwpool.tile([P, D], F16, name=f"wq_{k}")
                            for k in range(DC)]
                    for k in range(DC):
                        nc.sync.dma_start(wq_t[k], wq16[:, k * D:(k + 1) * D])
                    wlq_t = [fwpool.tile([P, D], F16, name=f"wlq_{k}")
                             for k in range(DC)]
                    wbq_t = [fwpool.tile([P, D], F16, name=f"wbq_{k}")
                             for k in range(DC)]
                    for k in range(DC):
                        nc.sync.dma_start(wlq_t[k], wlq16[:, k * D:(k + 1) * D])
                    for k in range(DC):
                        nc.sync.dma_start(wbq_t[k], wbq16[:, k * D:(k + 1) * D])

                    # scalar queue: Wk, Wv fp8 pairs, biases, Wa_q
                    wk_t = [xwpool.tile([P, 2, D], F8, name=f"wk_{kp}")
                            for kp in range(DP)]
                    wv_t = [xwpool.tile([P, 2, D], F8, name=f"wv_{kp}")
                            for kp in range(DP)]
                    for kp in range(DP):
                        nc.scalar.dma_start(
                            wk_t[kp],
                            wk8[:, kp * 2 * D:(kp + 1) * 2 * D]
                            .rearrange("p (j m) -> p j m", j=2))
                        nc.scalar.dma_start(
                            wv_t[kp],
                            wv8[:, kp * 2 * D:(kp + 1) * 2 * D]
                            .rearrange("p (j m) -> p j m", j=2))
                    nc.scalar.dma_start(bk_t, bk[:, :])
                    nc.scalar.dma_start(bvb_t, bvb[:, :])
                    nc.scalar.dma_start(bq_t, bq[:, :])
                    nc.scalar.dma_start(bl_t, bl[:, :])
                    nc.scalar.dma_start(ba_t, ba[:, :])
                    waq_t = [fwpool.tile([P, D], F16, name=f"waq_{k}")
                             for k in range(DC)]
                    for k in range(DC):
                        nc.scalar.dma_start(waq_t[k], waq16[:, k * D:(k + 1) * D])

                    # ============ K_c.T = Wk @ X_c.T + bk (fp8 DR) ============
                    kt_ps = [pspool.tile([P, R], F32, name=f"ktps{m}", tag="ps")
                             for m in range(DC)]
                    for kp in range(DP):
                        for m in range(DC):
                            nc.tensor.matmul(
                                kt_ps[m], wk_t[kp][:, :, m * P:(m + 1) * P],
                                x8_t[kp],
                                start=(kp == 0), stop=(kp == DP - 1),
                                perf_mode=DRM)
                    k8 = kvpool.tile([P, 4 * DC * P], F8, name="k8")
                    for m in range(DC):
                        nc.vector.tensor_scalar_add(
                            k8.rearrange("p (s m n) -> p s m n",
                                         s=4, m=DC)[:, :, m, :],
                            kt_ps[m].rearrange("p (s n) -> p s n", s=4),
                            bk_t[:, m:m + 1])
                    nc.scalar.dma_start(ktc_d[:, :], k8)
                    nc.gpsimd.collective_compute(
                        "AllGather", mybir.AluOpType.bypass, replica_groups=RG,
                        ins=[ktc_d.ap().opt()], outs=[ag_k.ap().opt()])

                    # ============ V_c = X_c @ Wv.T + bv (fp8 DR) ============
                    v_ps = [pspool.tile([P, R], F32, name=f"vps{i}", tag="ps")
                            for i in range(8)]
                    for kp in range(DP):
                        for rt in range(4):
                            for db in range(2):
                                nc.tensor.matmul(
                                    v_ps[rt * 2 + db],
                                    x8_t[kp][:, :, rt * P:(rt + 1) * P],
                                    wv_t[kp][:, :, db * 512:(db + 1) * 512],
                                    start=(kp == 0), stop=(kp == DP - 1),
                                    perf_mode=DRM)
                    v8 = kvpool.tile([P, 4 * D], F8, name="v8")
                    for rt in range(4):
                        for db in range(2):
                            nc.vector.tensor_add(
                                v8[:, rt * D + db * 512:rt * D + (db + 1) * 512],
                                v_ps[rt * 2 + db],
                                bvb_t[:, db * 512:(db + 1) * 512])
                    nc.scalar.dma_start(vc_d[:, :], v8)
                    nc.gpsimd.collective_compute(
                        "AllGather", mybir.AluOpType.bypass, replica_groups=RG,
                        ins=[vc_d.ap().opt()], outs=[ag_v.ap().opt()])

                    # ============ Q_c.T = Wq @ X_c.T + bq (fp16) ============
                    qt16 = [qpool.tile([P, R], F16, name=f"qt{m}")
                            for m in range(DC)]
                    q8p = [qpool.tile([P, 2, R], F8, name=f"q8p{mp}")
                           for mp in range(DP)]
                    q_ps = [pspool.tile([P, R], F32, name=f"qps{m}", tag="ps")
                            for m in range(DC)]
                    for k in range(DC):
                        for m in range(DC):
                            nc.tensor.matmul(
                                q_ps[m], wq_t[k][:, m * P:(m + 1) * P], xt_t[k],
                                start=(k == 0), stop=(k == DC - 1))
                    for m in range(DC):
                        nc.vector.tensor_scalar_add(
                            qt16[m], q_ps[m], bq_t[:, m:m + 1])
                        nc.vector.tensor_scalar_add(
                            q8p[m // 2][:, m % 2, :], q_ps[m], bq_t[:, m:m + 1])

                    # ---- gap fillers while the barrier + AGs complete ----
                    # Q-halves of x1 / h_a / h_b in fp16, biases folded in,
                    # partials staged to SBUF fp16.
                    fill_spec = [
                        ("x1q", wlq_t, bl_t, 0),
                        ("haq", waq_t, ba_t, 0),
                        ("hbq", wbq_t, ba_t, DC),
                    ]
                    fills = {}
                    for fname, wts, bias_t, bcol in fill_spec:
                        f_t = [qfpool.tile([P, R], F16, name=f"{fname}_{m}")
                               for m in range(DC)]
                        f_ps = [pspool.tile([P, R], F32, name=f"{fname}ps{m}",
                                            tag="ps") for m in range(DC)]
                        for k in range(DC):
                            for m in range(DC):
                                nc.tensor.matmul(
                                    f_ps[m], wts[k][:, m * P:(m + 1) * P],
                                    qt16[k],
                                    start=(k == 0), stop=(k == DC - 1))
                        for m in range(DC):
                            nc.vector.tensor_scalar_add(
                                f_t[m], f_ps[m],
                                bias_t[:, bcol + m:bcol + m + 1])
                        fills[fname] = f_t

                # ============ scoresT + exp + sums (fp8 DR) ============
                with tc.tile_pool(name="epool", bufs=1) as epool, \
                     tc.tile_pool(name="klpool", bufs=6) as klpool, \
                     tc.tile_pool(name="vlpool", bufs=1) as vlpool, \
                     tc.tile_pool(name="vtpool", bufs=1) as vtpool, \
                     tc.tile_pool(name="gwpool", bufs=1) as gwpool:
                    exp8 = [epool.tile([P, 2, R], F8, name=f"exp{t}")
                            for t in range(NKP)]
                    sums_ps = pspool.tile([1, R], F32, name="sums_ps", tag="ps")

                    def sums_mm(t):
                        nc.tensor.matmul(
                            sums_ps, ones8[:, :, 0:1], exp8[t],
                            start=(t == 0), stop=(t == NKP - 1),
                            perf_mode=DRM, skip_group_check=True)

                    # two score tiles in flight: consecutive matmuls hit
                    # independent PSUM banks so weight loads overlap compute
                    for t in range(NKP):
                        kls, scs = [], []
                        for j in range(2):
                            i = 2 * t + j
                            rr, ss = KT_ORDER[i]
                            kl8 = klpool.tile([P, DC, P], F8, name="kl8",
                                              tag="kl")
                            # the K-tile stream rides the scalar + gpsimd
                            # queues: the V gather starves the sync queue's
                            # DMA ring but not these two
                            eng = nc.scalar if j == 0 else nc.gpsimd
                            eng.dma_start(
                                kl8,
                                ag_k[rr * P:(rr + 1) * P,
                                     ss * DC * P:(ss + 1) * DC * P]
                                .rearrange("p (c n) -> p c n", n=P))
                            kls.append(kl8)
                            scs.append(pspool.tile([P, R], F32, name="sc_ps",
                                                   tag="ps"))
                        for mp in range(DP):
                            for j in range(2):
                                nc.tensor.matmul(
                                    scs[j], kls[j][:, 2 * mp:2 * mp + 2, :],
                                    q8p[mp],
                                    start=(mp == 0), stop=(mp == DP - 1),
                                    perf_mode=DRM)
                        for j in range(2):
                            nc.scalar.activation(
                                exp8[t][:, j, :], scs[j],
                                mybir.ActivationFunctionType.Exp,
                                bias=0.0, scale=1.0 / 32.0)
                        if t >= 1:
                            sums_mm(t - 1)   # one pair behind

                    # GLU fp8 weights stream in once the gathers are done
                    # with the wires (needed only at the final phase)
                    wlv_t = [gwpool.tile([P, 2, D], F8, name=f"wlv{mp}")
                             for mp in range(DP)]
                    wav_t = [gwpool.tile([P, 2, D], F8, name=f"wav{mp}")
                             for mp in range(DP)]
                    wbv_t = [gwpool.tile([P, 2, D], F8, name=f"wbv{mp}")
                             for mp in range(DP)]
                    for mp in range(DP):
                        for wt, wsrc in ((wlv_t, wlv8), (wav_t, wav8),
                                         (wbv_t, wbv8)):
                            nc.sync.dma_start(
                                wt[mp],
                                wsrc[:, mp * 2 * D:(mp + 1) * 2 * D]
                                .rearrange("p (j m) -> p j m", j=2))

                    # ============ V_T = (alpha @ V).T, two m-passes ============
                    vl8 = [vlpool.tile([P, 2, D], F8, name=f"vl8_{t}")
                           for t in range(NKP)]
                    vt_ps = [pspool.tile([P, R], F32, name=f"vtps{m}", tag="ps")
                             for m in range(4)]
                    recip_t = cpool.tile([1, R], F32, name="recip_t")
                    bc_t = cpool.tile([P, R], F32, name="bc_t")
                    for t in range(NKP):
                        for j in range(2):
                            rr, ss = KT_ORDER[2 * t + j]
                            eng = nc.sync if j == 0 else nc.gpsimd
                            eng.dma_start(
                                vl8[t][:, j, :],
                                ag_v[rr * P:(rr + 1) * P, ss * D:(ss + 1) * D])
                        for m in range(4):
                            nc.tensor.matmul(
                                vt_ps[m], vl8[t][:, :, m * P:(m + 1) * P],
                                exp8[t],
                                start=(t == 0), stop=(t == NKP - 1),
                                perf_mode=DRM, skip_group_check=True)
                        if t == 0:
                            sums_mm(NKP - 1)
                            nc.vector.reciprocal(recip_t, sums_ps)
                        if t == 8:
                            bc_ps = pspool.tile([P, R], F32, name="bc_ps",
                                                tag="ps")
                            nc.tensor.matmul(bc_ps, ones_row, recip_t,
                                             start=True, stop=True,
                                             skip_group_check=True)
                            nc.vector.tensor_copy(bc_t, bc_ps)

                    vt8 = [vtpool.tile([P, 2, R], F8, name=f"vt8_{mp}")
                           for mp in range(DP)]
                    vt_ps2 = [pspool.tile([P, R], F32, name=f"vtps2_{m}",
                                          tag="ps") for m in range(4)]
                    for t in range(NKP):
                        for m in range(4):
                            nc.tensor.matmul(
                                vt_ps2[m], vl8[t][:, :, (m + 4) * P:(m + 5) * P],
                                exp8[t],
                                start=(t == 0), stop=(t == NKP - 1),
                                perf_mode=DRM, skip_group_check=True)
                        if t == 0:
                            for m in range(4):   # normalize first half
                                nc.vector.tensor_mul(
                                    vt8[m // 2][:, m % 2, :], vt_ps[m], bc_t)
                    for m in range(4, DC):
                        nc.vector.tensor_mul(
                            vt8[m // 2][:, m % 2, :], vt_ps2[m - 4], bc_t)

                    # ============ V-halves of x1/h (fp8 DR) + GLU ============
                    with tc.tile_pool(name="fpool", bufs=1) as fpool:
                        for m in range(DC):
                            x1v_ps = pspool.tile([P, R], F32, name="x1v_ps",
                                                 tag="ps")
                            hav_ps = pspool.tile([P, R], F32, name="hav_ps",
                                                 tag="ps")
                            hbv_ps = pspool.tile([P, R], F32, name="hbv_ps",
                                                 tag="ps")
                            # interleave the three accumulation chains so
                            # consecutive matmuls are independent
                            for mp in range(DP):
                                for ps, wt in ((x1v_ps, wlv_t), (hav_ps, wav_t),
                                               (hbv_ps, wbv_t)):
                                    nc.tensor.matmul(
                                        ps, wt[mp][:, :, m * P:(m + 1) * P],
                                        vt8[mp], start=(mp == 0),
                                        stop=(mp == DP - 1), perf_mode=DRM)
                            # half-width fp16 tail: shorter serial chain after
                            # the last matmul, and output DMA starts at the
                            # first finished half
                            bt = fpool.tile([P, R], F16, name="bt", tag="bt",
                                            bufs=2)
                            sig = fpool.tile([P, R], F16, name="sig", tag="sig",
                                             bufs=2)
                            x1_t = fpool.tile([P, R], F16, name="x1_t", tag="x1",
                                              bufs=2)
                            a_t = fpool.tile([P, R], F16, name="a_t", tag="at",
                                             bufs=2)
                            o_t = fpool.tile([P, R], F32, name="o_t", tag="ot",
                                             bufs=2)
                            for hh in range(2):
                                sl = slice(hh * HR, (hh + 1) * HR)
                                nc.vector.tensor_add(
                                    bt[:, sl], hbv_ps[:, sl],
                                    fills["hbq"][m][:, sl])
                                nc.scalar.activation(
                                    sig[:, sl], bt[:, sl],
                                    mybir.ActivationFunctionType.Sigmoid,
                                    bias=0.0, scale=1.0)
                                nc.vector.tensor_add(
                                    x1_t[:, sl], x1v_ps[:, sl],
                                    fills["x1q"][m][:, sl])
                                nc.vector.tensor_add(
                                    a_t[:, sl], hav_ps[:, sl],
                                    fills["haq"][m][:, sl])
                                nc.vector.tensor_mul(
                                    a_t[:, sl], a_t[:, sl], sig[:, sl])
                                nc.vector.tensor_mul(
                                    o_t[:, sl], x1_t[:, sl], a_t[:, sl])
                                eng = nc.sync if (m + hh) % 2 == 0 else nc.scalar
                                eng.dma_start(
                                    out[m * P:(m + 1) * P, sl], o_t[:, sl])

    nc.compile()
    return nc


_NC = None


def _get_nc():
    global _NC
    if _NC is None:
        _NC = build_nc()
    return _NC


def _pair_layout(wT, dtype):
    """[K, C] (rows=contraction) -> [P, (K//256)*2*C] DoubleRow layout."""
    K, C = wT.shape
    a = wT.reshape(K // 256, 2, P, C).transpose(2, 0, 1, 3).reshape(P, -1)
    return np.ascontiguousarray(a.astype(dtype))


def _chunk_layout(wT, dtype):
    """[K, C] (rows=contraction) -> [P, (K//128)*C] chunk layout."""
    K, C = wT.shape
    a = wT.reshape(K // P, P, C).transpose(1, 0, 2).reshape(P, -1)
    return np.ascontiguousarray(a.astype(dtype))


def make_in_maps(input_features, Wq, bq, Wk, bk, Wv, bv, Wl, bl, Wa, ba):
    f = np.ascontiguousarray
    x = np.asarray(input_features, dtype=np.float32)
    xt_full = x.T                                          # [D, N] fp32
    wqT = np.asarray(Wq, np.float32).T
    wkT = np.asarray(Wk, np.float32).T
    wvT = np.asarray(Wv, np.float32).T
    wlT = np.asarray(Wl, np.float32).T                     # [2D, D]
    waT = np.asarray(Wa, np.float32).T                     # [2D, 2D]

    wq16 = _chunk_layout(wqT, np.float16)
    wk8 = _pair_layout(wkT, F8NP)
    wv8 = _pair_layout(wvT, F8NP)
    wlq16 = _chunk_layout(wlT[D:], np.float16)
    waq16 = _chunk_layout(waT[D:, :D], np.float16)
    wbq16 = _chunk_layout(waT[D:, D:], np.float16)
    wlv8 = _pair_layout(wlT[:D], F8NP)
    wav8 = _pair_layout(waT[:D, :D], F8NP)
    wbv8 = _pair_layout(waT[:D, D:], F8NP)

    bq_r = f(np.asarray(bq, np.float32).reshape(DC, P).T)  # [P, DC]
    bk_r = f(np.asarray(bk, np.float32).reshape(DC, P).T)
    bl_r = f(np.asarray(bl, np.float32).reshape(DC, P).T)
    ba_r = f(np.asarray(ba, np.float32).reshape(TDC, P).T)  # [P, TDC]
    bvb = f(np.broadcast_to(np.asarray(bv, np.float32), (P, D)))

    in_maps = []
    for c in range(NCORES):
        xt_c = xt_full[:, c * R:(c + 1) * R]               # [D, R]
        in_maps.append({
            "xt8": _pair_layout(xt_c, F8NP),
            "xt16": _chunk_layout(xt_c, np.float16),
            "wq16": wq16, "wk8": wk8, "wv8": wv8,
            "wlq16": wlq16, "waq16": waq16, "wbq16": wbq16,
            "wlv8": wlv8, "wav8": wav8, "wbv8": wbv8,
            "bq": bq_r, "bk": bk_r, "bvb": bvb, "bl": bl_r, "ba": ba_r,
        })
    return in_maps


def run(in_maps, trace=False):
    nc = _get_nc()
    return bass_utils.run_bass_kernel_spmd(
        nc, in_maps, core_ids=list(range(NCORES)), trace=trace)


def kernel(input_features, Wq, bq, Wk, bk, Wv, bv, Wl, bl, Wa, ba):
    in_maps = make_in_maps(input_features, Wq, bq, Wk, bk, Wv, bv, Wl, bl, Wa, ba)
    res = run(in_maps)
    out = np.empty((N, D), dtype=np.float32)
    for c in range(NCORES):
        out[c * R:(c + 1) * R, :] = res.results[c]["out"].T
    return out


# revision 38
# speedup vs baseline: 1.1812x; 1.1812x over previous
"""IntraAttention Trainium2 kernel, 8-core SPMD, mixed fp8/fp16.

Reference computation (N=4096 rows, d=1024):
    Q = X @ Wq.T + bq ; K = X @ Wk.T + bk ; V = X @ Wv.T + bv
    alpha = softmax(Q @ K.T / sqrt(d), axis=1)
    V_ = alpha @ V
    x = concat([V_, Q], axis=1)              # [N, 2d]
    x1 = x @ Wl.T + bl                        # [N, d]
    h = x @ Wa.T + ba                         # [N, 2d]
    out = x1 * (h[:, :d] * sigmoid(h[:, d:]))

Sharding: rows of X are sharded across 8 cores (512 rows each). Q stays
local; K and V shards are all-gathered in fp8 (one gather each --
every collective op carries ~8us fixed cost on the serial CC stream and
the whole stream is gated by a ~40-60us kernel-entry barrier that
absorbs cross-core launch skew). The Q-side fp16 projections (Q, and
the Q-halves of x1/h_a/h_b) run as gap fillers under that barrier+AG
window.

Precision: softmax averaging makes the whole attention path noise
immune, so K/V projections, scores, exp, and alpha@V run as fp8e4
DoubleRow matmuls (2x PE throughput, 256-deep contraction per
instruction). V_ is tiny relative to Q, so the V-halves of the
x1/h projections are fp8 DoubleRow too. Only the Q projection and the
Q-halves of x1/h_a/h_b stay fp16 (they dominate output accuracy).
All matmuls accumulate fp32 in PSUM.

DMA discipline: every weight class gets dedicated SBUF tiles whose
loads are all issued up front (the hardware DMA queues execute
in-order, so a single load waiting on a recycled buffer would block
everything queued behind it), queues are specialized (sync: X + fp16
weights + even K/V attention tiles; scalar: fp8 proj weights, biases,
collective staging, activations; gpsimd: collective triggers + odd
attention tiles), and all transfers move >=1KB contiguous per
partition.
"""

import numpy as np
import ml_dtypes

import concourse.bass as bass
import concourse.bacc as bacc
import concourse.tile as tile
import concourse.bass_utils as bass_utils
from concourse import mybir

P = 128            # partitions
D = 1024           # model dim
N = 4096           # rows
NCORES = 8
R = N // NCORES    # rows per core = 512
HR = R // 2        # half of the local rows = 256
DC = D // P        # d chunks = 8
DP = DC // 2       # d chunk pairs = 4
NK = N // P        # key tiles = 32
NKP = NK // 2      # key tile pairs = 16
TD = 2 * D         # 2048
TDC = TD // P      # 16

F32 = mybir.dt.float32
F16 = mybir.dt.float16
F8 = mybir.dt.float8e4
DRM = mybir.MatmulPerfMode.DoubleRow
F8NP = ml_dtypes.float8_e4m3

RG = [list(range(NCORES))]

# key-tile visit order; kt pairs (i=2t, 2t+1) pair exp slots with V rows
KT_ORDER = [(rr, ss) for ss in range(4) for rr in range(NCORES)]


def build_nc():
    nc = bacc.Bacc(
        "TRN2",
        target_bir_lowering=False,
        debug=False,
        num_devices=NCORES,
    )

    # ---- per-core I/O (host pre-arranged layouts, see make_in_maps) ----
    xt8 = nc.dram_tensor("xt8", [P, DP * 2 * R], F8, kind="ExternalInput")
    xt16 = nc.dram_tensor("xt16", [P, DC * R], F16, kind="ExternalInput")
    wq16 = nc.dram_tensor("wq16", [P, DC * D], F16, kind="ExternalInput")
    wk8 = nc.dram_tensor("wk8", [P, DP * 2 * D], F8, kind="ExternalInput")
    wv8 = nc.dram_tensor("wv8", [P, DP * 2 * D], F8, kind="ExternalInput")
    wlq16 = nc.dram_tensor("wlq16", [P, DC * D], F16, kind="ExternalInput")
    waq16 = nc.dram_tensor("waq16", [P, DC * D], F16, kind="ExternalInput")
    wbq16 = nc.dram_tensor("wbq16", [P, DC * D], F16, kind="ExternalInput")
    wlv8 = nc.dram_tensor("wlv8", [P, DP * 2 * D], F8, kind="ExternalInput")
    wav8 = nc.dram_tensor("wav8", [P, DP * 2 * D], F8, kind="ExternalInput")
    wbv8 = nc.dram_tensor("wbv8", [P, DP * 2 * D], F8, kind="ExternalInput")
    bq = nc.dram_tensor("bq", [P, DC], F32, kind="ExternalInput")
    bk = nc.dram_tensor("bk", [P, DC], F32, kind="ExternalInput")
    bvb = nc.dram_tensor("bvb", [P, D], F32, kind="ExternalInput")    # bv bcast
    bl = nc.dram_tensor("bl", [P, DC], F32, kind="ExternalInput")
    ba = nc.dram_tensor("ba", [P, TDC], F32, kind="ExternalInput")
    out = nc.dram_tensor("out", [D, R], F32, kind="ExternalOutput")   # out_c.T

    # ---- collective buffers (fp8), one gather each for K and V ----
    # K: [p, s, m, n] = K.T[d = m*128+p, key = s*128 + n (local)]
    ktc_d = nc.dram_tensor("ktc_d", [P, 4 * DC * P], F8)
    # V: [p, s, f] = V[row = s*128 + p (local), f]
    vc_d = nc.dram_tensor("vc_d", [P, 4 * D], F8)
    ag_k = nc.dram_tensor("ag_k", [NCORES * P, 4 * DC * P], F8,
                          addr_space="Shared")
    ag_v = nc.dram_tensor("ag_v", [NCORES * P, 4 * D], F8,
                          addr_space="Shared")

    with tile.TileContext(nc) as tc:
        with (
            tc.tile_pool(name="cpool", bufs=1) as cpool,
            tc.tile_pool(name="pspool", bufs=8, space="PSUM") as pspool,
        ):
            bq_t = cpool.tile([P, DC], F32, name="bq_t")
            bk_t = cpool.tile([P, DC], F32, name="bk_t")
            bl_t = cpool.tile([P, DC], F32, name="bl_t")
            ba_t = cpool.tile([P, TDC], F32, name="ba_t")
            bvb_t = cpool.tile([P, D], F32, name="bvb_t")
            # DoubleRow pair-dim stride must be a multiple of 16 elements,
            # so the ones column is padded to 16.
            ones8 = cpool.tile([P, 2, 16], F8, name="ones8")
            nc.vector.memset(ones8, 1.0)
            ones_row = cpool.tile([1, P], F32, name="ones_row")
            nc.vector.memset(ones_row, 1.0)

            with tc.tile_pool(name="qpool", bufs=1) as qpool, \
                 tc.tile_pool(name="qfpool", bufs=1) as qfpool, \
                 tc.tile_pool(name="kvpool", bufs=1) as kvpool:

                # ============ prefetches ============
                # All weight loads go to dedicated tiles and are issued up
                # front so the in-order DMA queues never head-of-line block.
                with tc.tile_pool(name="xwpool", bufs=1) as xwpool, \
                     tc.tile_pool(name="fwpool", bufs=1) as fwpool:
                    # sync queue: X fp8 pairs, X fp16 chunks, Wq, Wl_q, Wb_q
                    x8_t = [xwpool.tile([P, 2, R], F8, name=f"x8_{kp}")
                            for kp in range(DP)]
                    for kp in range(DP):
                        nc.sync.dma_start(
                            x8_t[kp],
                            xt8[:, kp * 2 * R:(kp + 1) * 2 * R]
                            .rearrange("p (j n) -> p j n", j=2))
                    xt_t = [xwpool.tile([P, R], F16, name=f"xt{k}")
                            for k in range(DC)]
                    for k in range(DC):
                        nc.sync.dma_start(xt_t[k], xt16[:, k * R:(k + 1) * R])
                    wq_t = [xwpool.tile([P, D], F16, name=f"wq_{k}")
                            for k in range(DC)]
                    for k in range(DC):
                        nc.sync.dma_start(wq_t[k], wq16[:, k * D:(k + 1) * D])
                    wlq_t = [fwpool.tile([P, D], F16, name=f"wlq_{k}")
                             for k in range(DC)]
                    wbq_t = [fwpool.tile([P, D], F16, name=f"wbq_{k}")
                             for k in range(DC)]
                    for k in range(DC):
                        nc.sync.dma_start(wlq_t[k], wlq16[:, k * D:(k + 1) * D])
                    for k in range(DC):
                        nc.sync.dma_start(wbq_t[k], wbq16[:, k * D:(k + 1) * D])

                    # scalar queue: Wk, Wv fp8 pairs, biases, Wa_q
                    wk_t = [xwpool.tile([P, 2, D], F8, name=f"wk_{kp}")
                            for kp in range(DP)]
                    wv_t = [xwpool.tile([P, 2, D], F8, name=f"wv_{kp}")
                            for kp in range(DP)]
                    for kp in range(DP):
                        nc.scalar.dma_start(
                            wk_t[kp],
                            wk8[:, kp * 2 * D:(kp + 1) * 2 * D]
                            .rearrange("p (j m) -> p j m", j=2))
                        nc.scalar.dma_start(
                            wv_t[kp],
                            wv8[:, kp * 2 * D:(kp + 1) * 2 * D]
                            .rearrange("p (j m) -> p j m", j=2))
                    nc.scalar.dma_start(bk_t, bk[:, :])
                    nc.scalar.dma_start(bvb_t, bvb[:, :])
                    nc.scalar.dma_start(bq_t, bq[:, :])
                    nc.scalar.dma_start(bl_t, bl[:, :])
                    nc.scalar.dma_start(ba_t, ba[:, :])
                    waq_t = [fwpool.tile([P, D], F16, name=f"waq_{k}")
                             for k in range(DC)]
                    for k in range(DC):
                        nc.scalar.dma_start(waq_t[k], waq16[:, k * D:(k + 1) * D])

                    # ============ K_c.T = Wk @ X_c.T + bk (fp8 DR) ============
                    kt_ps = [pspool.tile([P, R], F32, name=f"ktps{m}", tag="ps")
                             for m in range(DC)]
                    for kp in range(DP):
                        for m in range(DC):
                            nc.tensor.matmul(
                                kt_ps[m], wk_t[kp][:, :, m * P:(m + 1) * P],
                                x8_t[kp],
                                start=(kp == 0), stop=(kp == DP - 1),
                                perf_mode=DRM)
                    k8 = kvpool.tile([P, 4 * DC * P], F8, name="k8")
                    for m in range(DC):
                        nc.vector.tensor_scalar_add(
                            k8.rearrange("p (s m n) -> p s m n",
                                         s=4, m=DC)[:, :, m, :],
                            kt_ps[m].rearrange("p (s n) -> p s n", s=4),
                            bk_t[:, m:m + 1])
                    nc.scalar.dma_start(ktc_d[:, :], k8)
                    nc.gpsimd.collective_compute(
                        "AllGather", mybir.AluOpType.bypass, replica_groups=RG,
                        ins=[ktc_d.ap().opt()], outs=[ag_k.ap().opt()])

                    # ============ V_c = X_c @ Wv.T + bv (fp8 DR) ============
                    v_ps = [pspool.tile([P, R], F32, name=f"vps{i}", tag="ps")
                            for i in range(8)]
                    for kp in range(DP):
                        for rt in range(4):
                            for db in range(2):
                                nc.tensor.matmul(
                                    v_ps[rt * 2 + db],
                                    x8_t[kp][:, :, rt * P:(rt + 1) * P],
                                    wv_t[kp][:, :, db * 512:(db + 1) * 512],
                                    start=(kp == 0), stop=(kp == DP - 1),
                                    perf_mode=DRM)
                    v8 = kvpool.tile([P, 4 * D], F8, name="v8")
                    for rt in range(4):
                        for db in range(2):
                            nc.vector.tensor_add(
                                v8[:, rt * D + db * 512:rt * D + (db + 1) * 512],
                                v_ps[rt * 2 + db],
                                bvb_t[:, db * 512:(db + 1) * 512])
                    nc.scalar.dma_start(vc_d[:, :], v8)
                    nc.gpsimd.collective_compute(
                        "AllGather", mybir.AluOpType.bypass, replica_groups=RG,
                        ins=[vc_d.ap().opt()], outs=[ag_v.ap().opt()])

                    # ============ Q_c.T = Wq @ X_c.T + bq (fp16) ============
                    qt16 = [qpool.tile([P, R], F16, name=f"qt{m}")
                            for m in range(DC)]
                    q8p = [qpool.tile([P, 2, R], F8, name=f"q8p{mp}")
                           for mp in range(DP)]
                    q_ps = [pspool.tile([P, R], F32, name=f"qps{m}", tag="ps")
                            for m in range(DC)]
                    for k in range(DC):
                        for m in range(DC):
                            nc.tensor.matmul(
                                q_ps[m], wq_t[k][:, m * P:(m + 1) * P], xt_t[k],
                                start=(k == 0), stop=(k == DC - 1))
                    for m in range(DC):
                        nc.vector.tensor_scalar_add(
                            qt16[m], q_ps[m], bq_t[:, m:m + 1])
                        nc.vector.tensor_scalar_add(
                            q8p[m // 2][:, m % 2, :], q_ps[m], bq_t[:, m:m + 1])

                    # ---- gap fillers while the barrier + AGs complete ----
                    # Q-halves of x1 / h_a / h_b in fp16, biases folded in,
                    # partials staged to SBUF fp16.
                    fill_spec = [
                        ("x1q", wlq_t, bl_t, 0),
                        ("haq", waq_t, ba_t, 0),
                        ("hbq", wbq_t, ba_t, DC),
                    ]
                    fills = {}
                    for fname, wts, bias_t, bcol in fill_spec:
                        f_t = [qfpool.tile([P, R], F16, name=f"{fname}_{m}")
                               for m in range(DC)]
                        f_ps = [pspool.tile([P, R], F32, name=f"{fname}ps{m}",
                                            tag="ps") for m in range(DC)]
                        for k in range(DC):
                            for m in range(DC):
                                nc.tensor.matmul(
                                    f_ps[m], wts[k][:, m * P:(m + 1) * P],
                                    qt16[k],
                                    start=(k == 0), stop=(k == DC - 1))
                        for m in range(DC):
                            nc.vector.tensor_scalar_add(
                                f_t[m], f_ps[m],
                                bias_t[:, bcol + m:bcol + m + 1])
                        fills[fname] = f_t

                # ============ scoresT + exp + sums (fp8 DR) ============
                with tc.tile_pool(name="epool", bufs=1) as epool, \
                     tc.tile_pool(name="klpool", bufs=12) as klpool, \
                     tc.tile_pool(name="vlpool", bufs=1) as vlpool, \
                     tc.tile_pool(name="vtpool", bufs=1) as vtpool, \
                     tc.tile_pool(name="gwpool", bufs=1) as gwpool:
                    exp8 = [epool.tile([P, 2, R], F8, name=f"exp{t}")
                            for t in range(NKP)]
                    sums_ps = pspool.tile([1, R], F32, name="sums_ps", tag="ps")

                    def sums_mm(t):
                        nc.tensor.matmul(
                            sums_ps, ones8[:, :, 0:1], exp8[t],
                            start=(t == 0), stop=(t == NKP - 1),
                            perf_mode=DRM, skip_group_check=True)

                    # two score tiles in flight: consecutive matmuls hit
                    # independent PSUM banks so weight loads overlap compute
                    for t in range(NKP):
                        kls, scs = [], []
                        for j in range(2):
                            i = 2 * t + j
                            rr, ss = KT_ORDER[i]
                            kl8 = klpool.tile([P, DC, P], F8, name="kl8",
                                              tag="kl")
                            # first pairs ride the idle scalar queue so the
                            # first score matmul starts right as AG(K) lands
                            if t < 2:
                                eng = nc.scalar
                            else:
                                eng = nc.sync if j == 0 else nc.gpsimd
                            eng.dma_start(
                                kl8,
                                ag_k[rr * P:(rr + 1) * P,
                                     ss * DC * P:(ss + 1) * DC * P]
                                .rearrange("p (c n) -> p c n", n=P))
                            kls.append(kl8)
                            scs.append(pspool.tile([P, R], F32, name="sc_ps",
                                                   tag="ps"))
                        for mp in range(DP):
                            for j in range(2):
                                nc.tensor.matmul(
                                    scs[j], kls[j][:, 2 * mp:2 * mp + 2, :],
                                    q8p[mp],
                                    start=(mp == 0), stop=(mp == DP - 1),
                                    perf_mode=DRM)
                        for j in range(2):
                            nc.scalar.activation(
                                exp8[t][:, j, :], scs[j],
                                mybir.ActivationFunctionType.Exp,
                                bias=0.0, scale=1.0 / 32.0)
                        if t >= 1:
                            sums_mm(t - 1)   # one pair behind

                    # ============ V_T = (alpha @ V).T, two m-passes ============
                    vl8 = [vlpool.tile([P, 2, D], F8, name=f"vl8_{t}")
                           for t in range(NKP)]
                    vt_ps = [pspool.tile([P, R], F32, name=f"vtps{m}", tag="ps")
                             for m in range(4)]
                    recip_t = cpool.tile([1, R], F32, name="recip_t")
                    bc_t = cpool.tile([P, R], F32, name="bc_t")
                    for t in range(NKP):
                        for j in range(2):
                            rr, ss = KT_ORDER[2 * t + j]
                            eng = nc.sync if j == 0 else nc.gpsimd
                            eng.dma_start(
                                vl8[t][:, j, :],
                                ag_v[rr * P:(rr + 1) * P, ss * D:(ss + 1) * D])
                        for m in range(4):
                            nc.tensor.matmul(
                                vt_ps[m], vl8[t][:, :, m * P:(m + 1) * P],
                                exp8[t],
                                start=(t == 0), stop=(t == NKP - 1),
                                perf_mode=DRM, skip_group_check=True)
                        if t == 0:
                            sums_mm(NKP - 1)
                            nc.vector.reciprocal(recip_t, sums_ps)
                        if t == 8:
                            bc_ps = pspool.tile([P, R], F32, name="bc_ps",
                                                tag="ps")
                            nc.tensor.matmul(bc_ps, ones_row, recip_t,
                                             start=True, stop=True,
                                             skip_group_check=True)
                            nc.vector.tensor_copy(bc_t, bc_ps)

                    # GLU fp8 weights stream in behind the vl8 loads (sync
                    # queue is past its attention-tile work by now; needed
                    # only at the final phase)
                    wlv_t = [gwpool.tile([P, 2, D], F8, name=f"wlv{mp}")
                             for mp in range(DP)]
                    wav_t = [gwpool.tile([P, 2, D], F8, name=f"wav{mp}")
                             for mp in range(DP)]
                    wbv_t = [gwpool.tile([P, 2, D], F8, name=f"wbv{mp}")
                             for mp in range(DP)]
                    for mp in range(DP):
                        for wi, (wt, wsrc) in enumerate(
                                ((wlv_t, wlv8), (wav_t, wav8), (wbv_t, wbv8))):
                            eng = nc.sync if (mp * 3 + wi) % 2 == 0 else nc.scalar
                            eng.dma_start(
                                wt[mp],
                                wsrc[:, mp * 2 * D:(mp + 1) * 2 * D]
                                .rearrange("p (j m) -> p j m", j=2))

                    vt8 = [vtpool.tile([P, 2, R], F8, name=f"vt8_{mp}")
                           for mp in range(DP)]
                    vt_ps2 = [pspool.tile([P, R], F32, name=f"vtps2_{m}",
                                          tag="ps") for m in range(4)]
                    for t in range(NKP):
                        for m in range(4):
                            nc.tensor.matmul(
                                vt_ps2[m], vl8[t][:, :, (m + 4) * P:(m + 5) * P],
                                exp8[t],
                                start=(t == 0), stop=(t == NKP - 1),
                                perf_mode=DRM, skip_group_check=True)
                        if t == 0:
                            for m in range(4):   # normalize first half
                                nc.vector.tensor_mul(
                                    vt8[m // 2][:, m % 2, :], vt_ps[m], bc_t)
                    for m in range(4, DC):
                        nc.vector.tensor_mul(
                            vt8[m // 2][:, m % 2, :], vt_ps2[m - 4], bc_t)

                    # ============ V-halves of x1/h (fp8 DR) + GLU ============
                    with tc.tile_pool(name="fpool", bufs=1) as fpool:
                        for m in range(DC):
                            x1v_ps = pspool.tile([P, R], F32, name="x1v_ps",
                                                 tag="ps")
                            hav_ps = pspool.tile([P, R], F32, name="hav_ps",
                                                 tag="ps")
                            hbv_ps = pspool.tile([P, R], F32, name="hbv_ps",
                                                 tag="ps")
                            # interleave the three accumulation chains so
                            # consecutive matmuls are independent
                            for mp in range(DP):
                                for ps, wt in ((x1v_ps, wlv_t), (hav_ps, wav_t),
                                               (hbv_ps, wbv_t)):
                                    nc.tensor.matmul(
                                        ps, wt[mp][:, :, m * P:(m + 1) * P],
                                        vt8[mp], start=(mp == 0),
                                        stop=(mp == DP - 1), perf_mode=DRM)
                            # half-width fp16 tail: shorter serial chain after
                            # the last matmul, and output DMA starts at the
                            # first finished half
                            bt = fpool.tile([P, R], F16, name="bt", tag="bt",
                                            bufs=2)
                            sig = fpool.tile([P, R], F16, name="sig", tag="sig",
                                             bufs=2)
                            x1_t = fpool.tile([P, R], F16, name="x1_t", tag="x1",
                                              bufs=2)
                            a_t = fpool.tile([P, R], F16, name="a_t", tag="at",
                                             bufs=2)
                            o_t = fpool.tile([P, R], F32, name="o_t", tag="ot",
                                             bufs=2)
                            for hh in range(2):
                                sl = slice(hh * HR, (hh + 1) * HR)
                                nc.vector.tensor_add(
                                    bt[:, sl], hbv_ps[:, sl],
                                    fills["hbq"][m][:, sl])
                                nc.scalar.activation(
                                    sig[:, sl], bt[:, sl],
                                    mybir.ActivationFunctionType.Sigmoid,
                                    bias=0.0, scale=1.0)
                                nc.vector.tensor_add(
                                    x1_t[:, sl], x1v_ps[:, sl],
                                    fills["x1q"][m][:, sl])
                                nc.vector.tensor_add(
                                    a_t[:, sl], hav_ps[:, sl],
                                    fills["haq"][m][:, sl])
                                nc.vector.tensor_mul(
                                    a_t[:, sl], a_t[:, sl], sig[:, sl])
                                nc.vector.tensor_mul(
                                    o_t[:, sl], x1_t[:, sl], a_t[:, sl])
                                eng = nc.sync if (m + hh) % 2 == 0 else nc.scalar
                                eng.dma_start(
                                    out[m * P:(m + 1) * P, sl], o_t[:, sl])

    nc.compile()
    return nc


_NC = None


def _get_nc():
    global _NC
    if _NC is None:
        _NC = build_nc()
    return _NC


def _pair_layout(wT, dtype):
    """[K, C] (rows=contraction) -> [P, (K//256)*2*C] DoubleRow layout."""
    K, C = wT.shape
    a = wT.reshape(K // 256, 2, P, C).transpose(2, 0, 1, 3).reshape(P, -1)
    return np.ascontiguousarray(a.astype(dtype))


def _chunk_layout(wT, dtype):
    """[K, C] (rows=contraction) -> [P, (K//128)*C] chunk layout."""
    K, C = wT.shape
    a = wT.reshape(K // P, P, C).transpose(1, 0, 2).reshape(P, -1)
    return np.ascontiguousarray(a.astype(dtype))


def make_in_maps(input_features, Wq, bq, Wk, bk, Wv, bv, Wl, bl, Wa, ba):
    f = np.ascontiguousarray
    x = np.asarray(input_features, dtype=np.float32)
    xt_full = x.T                                          # [D, N] fp32
    wqT = np.asarray(Wq, np.float32).T
    wkT = np.asarray(Wk, np.float32).T
    wvT = np.asarray(Wv, np.float32).T
    wlT = np.asarray(Wl, np.float32).T                     # [2D, D]
    waT = np.asarray(Wa, np.float32).T                     # [2D, 2D]

    wq16 = _chunk_layout(wqT, np.float16)
    wk8 = _pair_layout(wkT, F8NP)
    wv8 = _pair_layout(wvT, F8NP)
    wlq16 = _chunk_layout(wlT[D:], np.float16)
    waq16 = _chunk_layout(waT[D:, :D], np.float16)
    wbq16 = _chunk_layout(waT[D:, D:], np.float16)
    wlv8 = _pair_layout(wlT[:D], F8NP)
    wav8 = _pair_layout(waT[:D, :D], F8NP)
    wbv8 = _pair_layout(waT[:D, D:], F8NP)

    bq_r = f(np.asarray(bq, np.float32).reshape(DC, P).T)  # [P, DC]
    bk_r = f(np.asarray(bk, np.float32).reshape(DC, P).T)
    bl_r = f(np.asarray(bl, np.float32).reshape(DC, P).T)
    ba_r = f(np.asarray(ba, np.float32).reshape(TDC, P).T)  # [P, TDC]
    bvb = f(np.broadcast_to(np.asarray(bv, np.float32), (P, D)))

    in_maps = []
    for c in range(NCORES):
        xt_c = xt_full[:, c * R:(c + 1) * R]               # [D, R]
        in_maps.append({
            "xt8": _pair_layout(xt_c, F8NP),
            "xt16": _chunk_layout(xt_c, np.float16),
            "wq16": wq16, "wk8": wk8, "wv8": wv8,
            "wlq16": wlq16, "waq16": waq16, "wbq16": wbq16,
            "wlv8": wlv8, "wav8": wav8, "wbv8": wbv8,
            "bq": bq_r, "bk": bk_r, "bvb": bvb, "bl": bl_r, "ba": ba_r,
        })
    return in_maps


def run(in_maps, trace=False):
    nc = _get_nc()
    return bass_utils.run_bass_kernel_spmd(
        nc, in_maps, core_ids=list(range(NCORES)), trace=trace)


def kernel(input_features, Wq, bq, Wk, bk, Wv, bv, Wl, bl, Wa, ba):
    in_maps = make_in_maps(input_features, Wq, bq, Wk, bk, Wv, bv, Wl, bl, Wa, ba)
    res = run(in_maps)
    out = np.empty((N, D), dtype=np.float32)
    for c in range(NCORES):
        out[c * R:(c + 1) * R, :] = res.results[c]["out"].T
    return out
